# revision 32
# baseline (speedup 1.0000x reference)
"""Trainium2 Bass kernel for the contextual channel-attention transformer block.

Contract: kernel(**inputs) takes the FULL unsharded inputs
(x: (8,512,64,64) f32, Wq/Wk/Wv: (512,512) f32, gamma: (1,) f32) and
returns the FULL (8,512,64,64) f32 output.  Internally the batch is
data-parallel across 8 NeuronCores (one batch element per core).

Per-core algorithm (fp8 e4m3 DoubleRow matmuls, fp32 PSUM):
  S    = X X^T / 64                     (64 DR MMs)
  M3q  = S Wq^T/8, M3k = S Wk^T/8      (16 DR MMs)
  G^T  = Wk M3q = 2 (Q K^T)^T          (8 DR MMs)
  |Q|^2, |K|^2 = colsum(W8 o M3) ones-matmuls; 1/sqrt via ACT Rsqrt.
  cos -> col-max -> temperature -> softmax on G^T[d,c]; rk folded into
  the exp scale; one fused tensor_tensor_reduce per d-group.
  A^T  = Wv^T Msm^T                     (8 DR MMs)
  r    = (A X) * gamma/rowsum           (64 DR MMs; row-L1 norm applied
  as a per-partition scale during PSUM evacuation), bf16 store.
  host computes y = x + r in fp32 (residual add off-device).

All DRAM tensors are host-packed to the SBUF tile layout so every DMA
is one contiguous descriptor per partition.  Warm-up matmuls during the
initial DMA wait and filler matmuls through the softmax keep the PE
HAM clock at 2.4 GHz.
"""

import os
import sys

for _p in ("/opt/trn_rl_repo", "/root/.axon_site/_ro/trn_rl_repo"):
    if os.path.isdir(_p) and _p not in sys.path:
        sys.path.insert(0, _p)

import ml_dtypes
import numpy as np

import concourse.bass as bass
import concourse.tile as tile
from concourse import bacc, bass_utils, mybir

B, C, HH, WW = 8, 512, 64, 64
N = HH * WW          # 4096 spatial positions
G = C // 128         # 4 channel groups of 128
N1 = N // 128        # 32 Gram chunks (128 spatial each)
NP = N1 // 2         # 16 chunk pairs for DoubleRow
NJ = N // 512        # 8 output chunks (512 spatial each)
EPS = 1e-6
INV_H = 4.0          # 1 / 0.25 temperature
FP32 = mybir.dt.float32
BF16 = mybir.dt.bfloat16
F8 = mybir.dt.float8e4
DR = mybir.MatmulPerfMode.DoubleRow

# feature flags for bisecting hardware-op support
F_TTR = os.environ.get("KF_TTR", "0") == "1"       # fused tensor_tensor_reduce (crashes NRT)
F_GPS_TT = os.environ.get("KF_GPS_TT", "1") == "1"  # gpsimd tensor_tensor fp8
F_GPS_APS = os.environ.get("KF_GPS_APS", "1") == "1"  # gpsimd AP scalars
F_RSQ = os.environ.get("KF_RSQ", "1") == "1"       # raw ACT Rsqrt
F_STT = os.environ.get("KF_STT", "1") == "1"       # scalar_tensor_tensor fuse
F_WARM = os.environ.get("KF_WARM", "1") == "1"     # PE warm-up/filler MMs

_CACHE = {}


def _act_raw(nc, out, in_, func, bias=0.0, scale=1.0, accum_out=None):
    """scalar.activation without the Rsqrt/Reciprocal accuracy guard."""
    se = nc.scalar
    if not isinstance(bias, float):
        pass
    elif func not in (mybir.ActivationFunctionType.Copy,
                      mybir.ActivationFunctionType.Reciprocal):
        bias = se.bass.const_aps.scalar_like(bias, in_)
    inputs = [se.lower_ap(in_)]
    for arg in (bias, scale, 0.0):
        if isinstance(arg, float):
            inputs.append(mybir.ImmediateValue(dtype=mybir.dt.float32,
                                               value=arg))
        else:
            inputs.append(se.lower_ap(arg))
    outputs = [se.lower_ap(out)]
    if accum_out is not None:
        outputs.append(se.lower_ap(accum_out))
    return se.add_instruction(
        mybir.InstActivation(
            name=se.bass.get_next_instruction_name(),
            func=func, ins=inputs, outs=outputs))


def _build_nc():
    nc = bacc.Bacc("TRN2", target_bir_lowering=False)

    xt8_d = nc.dram_tensor("xt8", [128, N1 * C], F8, kind="ExternalInput")
    xh8_d = nc.dram_tensor("xh8", [128, G * N], F8, kind="ExternalInput")
    w8_d = nc.dram_tensor("w8", [128, 3 * G * C], F8, kind="ExternalInput")
    gcol_d = nc.dram_tensor("gamma_col", [128, 1], FP32, kind="ExternalInput")
    obf_d = nc.dram_tensor("ones_bf", [128, 1], BF16, kind="ExternalInput")
    orow_d = nc.dram_tensor("ones_row", [1, 128], BF16, kind="ExternalInput")
    o8_d = nc.dram_tensor("ones_f8", [128, 1], F8, kind="ExternalInput")
    y_d = nc.dram_tensor("y", [128, G * N], F8, kind="ExternalOutput")

    xt_v = xt8_d.ap().rearrange("p (i c) -> p i c", c=C)      # [128, N1, C]
    xh_v = xh8_d.ap().rearrange("p (g n) -> p g n", n=N)      # [128, G, N]
    w_v = w8_d.ap().rearrange("p (t g o) -> p t g o", t=3, o=C)
    y_v = y_d.ap().rearrange("p (g n) -> p g n", n=N)

    MUL = mybir.AluOpType.mult
    ADD = mybir.AluOpType.add
    MIN = mybir.AluOpType.min
    Exp = mybir.ActivationFunctionType.Exp
    Sqrt = mybir.ActivationFunctionType.Sqrt
    Rsqrt = mybir.ActivationFunctionType.Rsqrt
    Copy = mybir.ActivationFunctionType.Copy

    with tile.TileContext(nc) as tc:
        with (
            tc.tile_pool(name="consts", bufs=1) as cpool,
            tc.tile_pool(name="weights", bufs=1) as wpool,
            tc.tile_pool(name="xbig", bufs=1) as xpool,
            tc.tile_pool(name="inter8", bufs=1) as ipool,
            tc.tile_pool(name="norm", bufs=1) as npool,
            tc.tile_pool(name="smx", bufs=2) as spool,
            tc.tile_pool(name="small", bufs=2) as qpool,
            tc.tile_pool(name="outs", bufs=2) as opool,
            tc.tile_pool(name="ps", bufs=1, space="PSUM") as ps,
        ):
            # one PSUM tile per bank; phases reuse banks via slices with
            # natural data-dependency slack.
            bk = [ps.tile([128, 512], FP32, tag=f"bk{i}", name=f"bk{i}")
                  for i in range(8)]

            # ---- scratch + ACT table preloads (run during DMA wait) ----
            scr = cpool.tile([1, 4], FP32, tag="scr")
            warm8 = None
            with tc.high_priority():
                if F_WARM:
                    warm8 = cpool.tile([128, 2, 512], F8, tag="warm8")
                    nc.vector.memset(warm8[:], 0.0)
                nc.gpsimd.memset(scr[:], 1.0)
                if F_RSQ:
                    _act_raw(nc, scr[:, 1:2], scr[:, 0:1], Rsqrt)
                else:
                    nc.scalar.activation(scr[:, 1:2], scr[:, 0:1], Sqrt)
                _act_raw(nc, scr[:, 2:3], scr[:, 0:1], Exp)

            # ---- input DMAs: xt8 first (Gx critical path) --------------
            xt8 = xpool.tile([128, N1, C], F8, tag="xt8")
            for s in range(8):
                nc.sync.dma_start(xt8[:, 4 * s:4 * s + 4, :],
                                  xt_v[:, 4 * s:4 * s + 4, :])
            w8 = wpool.tile([128, 3, G, C], F8, tag="w8")
            nc.sync.dma_start(w8[:], w_v)
            wq8, wk8, wv8 = w8[:, 0], w8[:, 1], w8[:, 2]
            gamma_col = cpool.tile([128, 1], FP32, tag="gamma_col")
            nc.sync.dma_start(gamma_col[:], gcol_d.ap())
            ones_bf = cpool.tile([128, 1], BF16, tag="ones_bf")
            nc.sync.dma_start(ones_bf[:], obf_d.ap())
            ones_row = cpool.tile([1, 128], BF16, tag="ones_row")
            nc.sync.dma_start(ones_row[:], orow_d.ap())
            ones8 = cpool.tile([128, 1], F8, tag="ones8")
            nc.sync.dma_start(ones8[:], o8_d.ap())
            xh8 = xpool.tile([128, G, N], F8, tag="xh8")
            nc.sync.dma_start(xh8[:, 0:2, :], xh_v[:, 0:2, :])
            nc.sync.dma_start(xh8[:, 2:4, :], xh_v[:, 2:4, :])

            # ---- PE warm-up during DMA wait (bank 7, garbage output) ---
            if F_WARM:
                for w in range(8):
                    nc.tensor.matmul(bk[7][:], warm8[:, :, 0:128],
                                     warm8[:], start=True, stop=True,
                                     perf_mode=DR)

            # ---- Gx = X X^T (fp8 DR, PSUM-accumulated) banks 0-3 -------
            gx8 = ipool.tile([128, G, C], F8, tag="gx8")
            for i in range(NP):
                for cg in range(G):
                    nc.tensor.matmul(bk[cg][:],
                                     xt8[:, 2 * i:2 * i + 2,
                                         cg * 128:(cg + 1) * 128],
                                     xt8[:, 2 * i:2 * i + 2, :],
                                     start=(i == 0), stop=(i == NP - 1),
                                     perf_mode=DR)
            for cg in range(G):
                if cg % 2:
                    nc.vector.tensor_scalar(gx8[:, cg, :], bk[cg][:],
                                            1.0 / 64.0, None, op0=MUL)
                else:
                    nc.scalar.activation(gx8[:, cg, :], bk[cg][:],
                                         Copy, scale=1.0 / 64.0)

            # ---- M3q = S Wq^T (banks 4-7), M3k = S Wk^T (banks 0-3) ----
            m3q8 = ipool.tile([128, G, C], F8, tag="m3q8")
            m3k8 = None
            if not F_STT:
                m3k8 = ipool.tile([128, G, C], F8, tag="m3k8")
            # interleave M3q pair / q-cast / M3k pair per cg so each cast
            # fires right after its producer in the PE stream
            for cg in range(G):
                for p in range(2):
                    nc.tensor.matmul(bk[4 + cg][:],
                                     gx8[:, 2 * p:2 * p + 2,
                                         cg * 128:(cg + 1) * 128],
                                     wq8[:, 2 * p:2 * p + 2, :],
                                     start=(p == 0), stop=(p == 1),
                                     perf_mode=DR)
                if cg % 2:
                    nc.vector.tensor_scalar(m3q8[:, cg, :], bk[4 + cg][:],
                                            0.5, None, op0=MUL)
                else:
                    nc.scalar.activation(m3q8[:, cg, :], bk[4 + cg][:],
                                         Copy, scale=0.5)
                for p in range(2):
                    nc.tensor.matmul(bk[cg][:],
                                     gx8[:, 2 * p:2 * p + 2,
                                         cg * 128:(cg + 1) * 128],
                                     wk8[:, 2 * p:2 * p + 2, :],
                                     start=(p == 0), stop=(p == 1),
                                     perf_mode=DR)

            # ---- norms: tq/tk = W8 o M3 (2|Q|^2 / 2|K|^2 colsums) ------
            # tk fused from k_ps PSUM via scalar_tensor_tensor (no m3k8
            # fp8 cast at all); tq g<2 from q_ps PSUM, g>=2 from m3q8.
            tq = npool.tile([128, G, C], BF16, tag="tq")
            tk = npool.tile([128, G, C], BF16, tag="tk")
            for g in range(G):
                if F_STT and g < 2:
                    nc.vector.scalar_tensor_tensor(
                        tq[:, g, :], bk[4 + g][:], 0.5, wq8[:, g, :],
                        op0=MUL, op1=MUL)
                elif F_GPS_TT and g == 3:
                    nc.gpsimd.tensor_tensor(tq[:, g, :], wq8[:, g, :],
                                            m3q8[:, g, :], op=MUL)
                else:
                    nc.vector.tensor_tensor(tq[:, g, :], wq8[:, g, :],
                                            m3q8[:, g, :], op=MUL)
            for g in range(G):
                if F_STT:
                    nc.vector.scalar_tensor_tensor(
                        tk[:, g, :], bk[g][:], 0.5, wk8[:, g, :],
                        op0=MUL, op1=MUL)
                else:
                    nc.vector.tensor_scalar(m3k8[:, g, :], bk[g][:],
                                            0.5, None, op0=MUL)
                    nc.vector.tensor_tensor(tk[:, g, :], wk8[:, g, :],
                                            m3k8[:, g, :], op=MUL)

            # ---- G^T per d-group (banks 4-7, alive through softmax) ----
            # dg 2,3 first: their banks have no pending tq readers.
            for dg in (2, 3, 0, 1):
                for p in range(2):
                    nc.tensor.matmul(
                        bk[4 + dg][:],
                        wk8[:, 2 * p:2 * p + 2, dg * 128:(dg + 1) * 128],
                        m3q8[:, 2 * p:2 * p + 2, :],
                        start=(p == 0), stop=(p == 1), perf_mode=DR)

            # sqq = colsum(tq) -> bank 0 row 0 (free axis = q-channel)
            for g in range(G):
                nc.tensor.matmul(bk[0][0:1, :], ones_bf[:], tq[:, g, :],
                                 start=(g == 0), stop=(g == G - 1))
            # sqk[dg] -> banks 1,2,3 col 0 and bank 0 col 4
            sqk_bank = [1, 2, 3, 0]
            sqk_col = [0, 0, 0, 4]
            for dg in range(G):
                b, cl = sqk_bank[dg], sqk_col[dg]
                for g in range(G):
                    nc.tensor.matmul(bk[b][:, cl:cl + 1],
                                     tk[:, g, dg * 128:(dg + 1) * 128],
                                     ones_bf[:],
                                     start=(g == 0), stop=(g == G - 1))

            # rq = 0.5/|Q| = Rsqrt(2*sqq); rk = 1/|K| = Rsqrt(0.5*sqk)
            rq_bf = npool.tile([1, C], BF16, tag="rq_bf")
            rk4 = npool.tile([128, 4], FP32, tag="rk4")
            if F_RSQ:
                _act_raw(nc, rq_bf[:], bk[0][0:1, :], Rsqrt, scale=2.0)
                for dg in range(G):
                    b, cl = sqk_bank[dg], sqk_col[dg]
                    _act_raw(nc, rk4[:, dg:dg + 1], bk[b][:, cl:cl + 1],
                             Rsqrt, scale=0.5)
            else:
                invq = npool.tile([1, C], FP32, tag="invq")
                nc.vector.reciprocal(invq[:], bk[0][0:1, :])
                nc.scalar.activation(rq_bf[:], invq[:], Sqrt, scale=0.5)
                invk4 = npool.tile([128, 4], FP32, tag="invk4")
                for dg in range(G):
                    b, cl = sqk_bank[dg], sqk_col[dg]
                    nc.vector.reciprocal(invk4[:, dg:dg + 1],
                                         bk[b][:, cl:cl + 1])
                nc.scalar.activation(rk4[:], invk4[:], Sqrt, scale=2.0)
            nrk4 = npool.tile([128, 4], FP32, tag="nrk4")
            nc.vector.tensor_scalar(nrk4[:], rk4[:], -1.0, None, op0=MUL)
            rk4h = npool.tile([128, 4], FP32, tag="rk4h")
            nc.vector.tensor_scalar(rk4h[:], rk4[:], INV_H, None, op0=MUL)

            # pinned dummy exp: forces the Exp table load into the ACT
            # idle window before the real exps need it
            with tc.tile_wait_until(0.024):
                _act_raw(nc, scr[:, 3:4], scr[:, 0:1], Exp)

            # bq = broadcast(rq) via ones-column matmul -> bank 1
            nc.tensor.matmul(bk[1][:], ones_row[:], rq_bf[:],
                             start=True, stop=True)
            bq = npool.tile([128, C], BF16, tag="bq")
            nc.scalar.copy(bq[:], bk[1][:])

            # warm burst: PE is idle through the softmax window; ~14 MMs
            # of garbage work keep HAM at 2.4 GHz so A^T/phase2 start warm
            if F_WARM:
                for w in range(14):
                    nc.tensor.matmul(bk[2][:], warm8[:, :, 0:128],
                                     warm8[:], start=True, stop=True,
                                     perf_mode=DR)

            # filler matmul: keep HAM warm between bq and A^T (bank 2)
            if F_WARM:
                nc.tensor.matmul(bk[2][:], warm8[:, :, 0:128], warm8[:],
                                 start=True, stop=True, perf_mode=DR)

            # ---- softmax chains on G^T[d,c] + A^T accumulation ---------
            msm8 = ipool.tile([128, G, C], F8, tag="msm8")
            at8 = ipool.tile([128, G, C], F8, tag="at8")
            at_banks = [2, 3, 0, 1]
            for dg in range(G):
                t1 = spool.tile([128, C], BF16, tag="t1")
                mn = qpool.tile([128, 1], FP32, tag="mn")
                if F_TTR:
                    nc.vector.tensor_tensor_reduce(
                        t1[:], bq[:], bk[4 + dg][:], 1.0, 0.0,
                        op0=MUL, op1=MIN, accum_out=mn[:])
                else:
                    nc.vector.tensor_tensor(t1[:], bk[4 + dg][:], bq[:],
                                            op=MUL)
                    nc.vector.tensor_reduce(mn[:], t1[:],
                                            axis=mybir.AxisListType.X, op=MIN)
                den = qpool.tile([128, 1], FP32, tag="den")
                eng = nc.gpsimd if F_GPS_APS else nc.vector
                eng.tensor_scalar(den[:], mn[:], nrk4[:, dg:dg + 1],
                                  1.0 + EPS, op0=MUL, op1=ADD)
                r = qpool.tile([128, 1], FP32, tag="r")
                nc.vector.reciprocal(r[:], den[:])
                sv = qpool.tile([128, 1], FP32, tag="sv")
                eng.tensor_scalar(sv[:], r[:], rk4h[:, dg:dg + 1],
                                  None, op0=MUL)
                bv = qpool.tile([128, 1], FP32, tag="bv")
                nc.gpsimd.tensor_scalar(bv[:], r[:], -INV_H, 1.0,
                                        op0=MUL, op1=ADD)
                e = spool.tile([128, C], BF16, tag="e")
                se = qpool.tile([128, 1], FP32, tag="se")
                nc.scalar.activation(e[:], t1[:], Exp,
                                     bias=bv[:], scale=sv[:],
                                     accum_out=se[:])
                rd = qpool.tile([128, 1], FP32, tag="rd")
                nc.vector.reciprocal(rd[:], se[:])
                if dg % 2:
                    # ACT path: msm = Copy(e * rd64), rd64 on gpsimd
                    rd64 = qpool.tile([128, 1], FP32, tag="rd64")
                    nc.gpsimd.tensor_scalar(rd64[:], rd[:], 64.0, None,
                                            op0=MUL)
                    nc.scalar.activation(msm8[:, dg, :], e[:], Copy,
                                         scale=rd64[:])
                else:
                    nc.vector.tensor_scalar(msm8[:, dg, :], e[:], rd[:],
                                            64.0, op0=MUL, op1=MUL)
                if F_WARM and dg == 0:
                    nc.tensor.matmul(bk[3][:], wv8[:, 0, 0:128],
                                     msm8[:, 0, :], start=True, stop=True)
                # A^T accumulation over dg pairs into banks 2,3,0,1
                if dg % 2:
                    p = dg // 2
                    for eg in range(G):
                        nc.tensor.matmul(
                            bk[at_banks[eg]][:],
                            wv8[:, dg - 1:dg + 1,
                                eg * 128:(eg + 1) * 128],
                            msm8[:, dg - 1:dg + 1, :],
                            start=(p == 0), stop=(p == 1), perf_mode=DR)

            # ---- row-L1 norm -> per-channel scale fcol (banks 4-7) -----
            # rsum[c] = 64 * sum_d Msm[c,d]; fcol = gamma / (32*(rsum+64eps))
            for cg in range(G):
                for g in range(G):
                    nc.tensor.matmul(bk[4 + cg][:, 0:1],
                                     msm8[:, g, cg * 128:(cg + 1) * 128],
                                     ones8[:],
                                     start=(g == 0), stop=(g == G - 1))
            eps4 = npool.tile([128, 4], FP32, tag="eps4")
            for cg in range(G):
                nc.vector.tensor_scalar(eps4[:, cg:cg + 1],
                                        bk[4 + cg][:, 0:1],
                                        64.0 * EPS, None, op0=ADD)
            invr4 = npool.tile([128, 4], FP32, tag="invr4")
            nc.vector.reciprocal(invr4[:], eps4[:])
            fcol4 = npool.tile([128, 4], FP32, tag="fcol4")
            eng = nc.gpsimd if F_GPS_APS else nc.vector
            eng.tensor_scalar(fcol4[:], invr4[:], gamma_col[:],
                              None, op0=MUL)

            # at8 = 2 * at_ps (fp8 cast; scale keeps absmax ~60)
            for eg in range(G):
                if eg % 2:
                    nc.vector.tensor_scalar(at8[:, eg, :],
                                            bk[at_banks[eg]][:],
                                            2.0, None, op0=MUL)
                else:
                    nc.scalar.activation(at8[:, eg, :], bk[at_banks[eg]][:],
                                         Copy, scale=2.0)

            # ---- phase 2: r = (A X) * fcol, bf16 store -----------------
            for cg in range(G):
                ofin = opool.tile([128, N], F8, tag="ofin",
                                  name=f"ofin{cg}")
                for p in range(2):
                    lhs = at8[:, 2 * p:2 * p + 2,
                              cg * 128:(cg + 1) * 128]
                    for j in range(NJ):
                        nc.tensor.matmul(bk[j][:], lhs,
                                         xh8[:, 2 * p:2 * p + 2,
                                             j * 512:(j + 1) * 512],
                                         start=(p == 0), stop=(p == 1),
                                         perf_mode=DR)
                for j in range(NJ):
                    jsl = slice(j * 512, (j + 1) * 512)
                    if j % 2:
                        nc.vector.tensor_scalar(ofin[:, jsl], bk[j][:],
                                                fcol4[:, cg:cg + 1],
                                                None, op0=MUL)
                    else:
                        nc.scalar.activation(ofin[:, jsl], bk[j][:],
                                             Copy, scale=fcol4[:, cg:cg + 1])
                    if j == 3:
                        nc.sync.dma_start(y_v[:, cg, 0:2048],
                                          ofin[:, 0:2048])
                    if cg == G - 1 and j == 5:
                        nc.sync.dma_start(y_v[:, cg, 2048:3072],
                                          ofin[:, 2048:3072])
                if cg == G - 1:
                    nc.sync.dma_start(y_v[:, cg, 3072:4096],
                                      ofin[:, 3072:4096])
                else:
                    nc.sync.dma_start(y_v[:, cg, 2048:4096],
                                      ofin[:, 2048:4096])

    nc.compile()
    return nc


def _get_nc():
    if "nc" not in _CACHE:
        _CACHE["nc"] = _build_nc()
    return _CACHE["nc"]


def _pack_rows(a):
    """[C, M] -> [128, G*M] with row r = g*128+p at [p, g*M:(g+1)*M]."""
    Cr, M = a.shape
    return np.ascontiguousarray(
        a.reshape(Cr // 128, 128, M).transpose(1, 0, 2).reshape(128, -1))


def _make_in_maps(x, Wq, Wk, Wv, gamma):
    F8NP = ml_dtypes.float8_e4m3
    xb = np.ascontiguousarray(x.reshape(B, C, N).astype(np.float32))
    xh8 = xb.astype(F8NP)
    xt8 = np.ascontiguousarray(xb.transpose(0, 2, 1)).astype(F8NP)
    # pack to [128, *] SBUF layouts (one contiguous line per partition)
    xt8p = np.ascontiguousarray(
        xt8.reshape(B, N1, 128, C).transpose(0, 2, 1, 3).reshape(B, 128, -1))
    xh8p = np.ascontiguousarray(
        xh8.reshape(B, G, 128, N).transpose(0, 2, 1, 3).reshape(B, 128, -1))
    wq8 = _pack_rows((np.ascontiguousarray(Wq.T) * 16.0).astype(F8NP))
    wk8 = _pack_rows((np.ascontiguousarray(Wk.T) * 16.0).astype(F8NP))
    wv8 = _pack_rows((np.asarray(Wv) * 16.0).astype(F8NP))
    w8 = np.ascontiguousarray(np.concatenate([wq8, wk8, wv8], axis=1))
    gval = float(np.asarray(gamma).reshape(-1)[0])
    gcol = np.full((128, 1), gval / 2.0, np.float32)  # gamma/32 * 16 (fp8 out scale)
    obf = np.ones((128, 1), ml_dtypes.bfloat16)
    orow = np.ones((1, 128), ml_dtypes.bfloat16)
    o8 = np.ones((128, 1), F8NP)
    maps = []
    for i in range(B):
        maps.append({
            "xt8": xt8p[i], "xh8": xh8p[i], "w8": w8,
            "gamma_col": gcol, "ones_bf": obf, "ones_row": orow,
            "ones_f8": o8,
        })
    return maps


def kernel(x, Wq, Wk, Wv, gamma, _trace=False, _trace_kwargs=None):
    nc = _get_nc()
    xnp = np.asarray(x)
    in_maps = _make_in_maps(xnp, np.asarray(Wq), np.asarray(Wk),
                            np.asarray(Wv), np.asarray(gamma))
    kwargs = {}
    if _trace:
        kwargs = dict(trace=True, **(_trace_kwargs or {}))
    res = bass_utils.run_bass_kernel_spmd(nc, in_maps,
                                          core_ids=list(range(B)), **kwargs)
    # y [128, G*N] -> [C, N]
    r = np.stack([
        res.results[i]["y"].reshape(128, G, N).transpose(1, 0, 2)
        .reshape(C, N).astype(np.float32) for i in range(B)]) * (1.0 / 16.0)
    y = xnp.reshape(B, C, N).astype(np.float32) + r
    if _trace:
        kernel._last_result = res
    return y.reshape(B, C, HH, WW).astype(np.float32)


# revision 33
# speedup vs baseline: 1.0140x; 1.0140x over previous
"""Trainium2 Bass kernel for the contextual channel-attention transformer block.

Contract: kernel(**inputs) takes the FULL unsharded inputs
(x: (8,512,64,64) f32, Wq/Wk/Wv: (512,512) f32, gamma: (1,) f32) and
returns the FULL (8,512,64,64) f32 output.  Internally the batch is
data-parallel across 8 NeuronCores (one batch element per core).

Per-core algorithm (fp8 e4m3 DoubleRow matmuls, fp32 PSUM):
  S    = X X^T / 64                     (64 DR MMs)
  M3q  = S Wq^T/8, M3k = S Wk^T/8      (16 DR MMs)
  G^T  = Wk M3q = 2 (Q K^T)^T          (8 DR MMs)
  |Q|^2, |K|^2 = colsum(W8 o M3) ones-matmuls; 1/sqrt via ACT Rsqrt.
  cos -> col-max -> temperature -> softmax on G^T[d,c]; rk folded into
  the exp scale; one fused tensor_tensor_reduce per d-group.
  A^T  = Wv^T Msm^T                     (8 DR MMs)
  r    = (A X) * gamma/rowsum           (64 DR MMs; row-L1 norm applied
  as a per-partition scale during PSUM evacuation), bf16 store.
  host computes y = x + r in fp32 (residual add off-device).

All DRAM tensors are host-packed to the SBUF tile layout so every DMA
is one contiguous descriptor per partition.  Warm-up matmuls during the
initial DMA wait and filler matmuls through the softmax keep the PE
HAM clock at 2.4 GHz.
"""

import os
import sys

for _p in ("/opt/trn_rl_repo", "/root/.axon_site/_ro/trn_rl_repo"):
    if os.path.isdir(_p) and _p not in sys.path:
        sys.path.insert(0, _p)

import ml_dtypes
import numpy as np

import concourse.bass as bass
import concourse.tile as tile
from concourse import bacc, bass_utils, mybir

B, C, HH, WW = 8, 512, 64, 64
N = HH * WW          # 4096 spatial positions
G = C // 128         # 4 channel groups of 128
N1 = N // 128        # 32 Gram chunks (128 spatial each)
NP = N1 // 2         # 16 chunk pairs for DoubleRow
NJ = N // 512        # 8 output chunks (512 spatial each)
EPS = 1e-6
INV_H = 4.0          # 1 / 0.25 temperature
FP32 = mybir.dt.float32
BF16 = mybir.dt.bfloat16
F8 = mybir.dt.float8e4
DR = mybir.MatmulPerfMode.DoubleRow

# feature flags for bisecting hardware-op support
F_TTR = os.environ.get("KF_TTR", "0") == "1"       # fused tensor_tensor_reduce (crashes NRT)
F_GPS_TT = os.environ.get("KF_GPS_TT", "1") == "1"  # gpsimd tensor_tensor fp8
F_GPS_APS = os.environ.get("KF_GPS_APS", "1") == "1"  # gpsimd AP scalars
F_RSQ = os.environ.get("KF_RSQ", "1") == "1"       # raw ACT Rsqrt
F_STT = os.environ.get("KF_STT", "1") == "1"       # scalar_tensor_tensor fuse
F_WARM = os.environ.get("KF_WARM", "1") == "1"     # PE warm-up/filler MMs

_CACHE = {}


def _act_raw(nc, out, in_, func, bias=0.0, scale=1.0, accum_out=None):
    """scalar.activation without the Rsqrt/Reciprocal accuracy guard."""
    se = nc.scalar
    if not isinstance(bias, float):
        pass
    elif func not in (mybir.ActivationFunctionType.Copy,
                      mybir.ActivationFunctionType.Reciprocal):
        bias = se.bass.const_aps.scalar_like(bias, in_)
    inputs = [se.lower_ap(in_)]
    for arg in (bias, scale, 0.0):
        if isinstance(arg, float):
            inputs.append(mybir.ImmediateValue(dtype=mybir.dt.float32,
                                               value=arg))
        else:
            inputs.append(se.lower_ap(arg))
    outputs = [se.lower_ap(out)]
    if accum_out is not None:
        outputs.append(se.lower_ap(accum_out))
    return se.add_instruction(
        mybir.InstActivation(
            name=se.bass.get_next_instruction_name(),
            func=func, ins=inputs, outs=outputs))


def _build_nc():
    nc = bacc.Bacc("TRN2", target_bir_lowering=False)

    xt8_d = nc.dram_tensor("xt8", [128, N1 * C], F8, kind="ExternalInput")
    xh8_d = nc.dram_tensor("xh8", [128, G * N], F8, kind="ExternalInput")
    w8_d = nc.dram_tensor("w8", [128, 3 * G * C], F8, kind="ExternalInput")
    gcol_d = nc.dram_tensor("gamma_col", [128, 1], FP32, kind="ExternalInput")
    obf_d = nc.dram_tensor("ones_bf", [128, 1], BF16, kind="ExternalInput")
    orow_d = nc.dram_tensor("ones_row", [1, 128], BF16, kind="ExternalInput")
    o8_d = nc.dram_tensor("ones_f8", [128, 1], F8, kind="ExternalInput")
    y_d = nc.dram_tensor("y", [128, G * N], F8, kind="ExternalOutput")

    xt_v = xt8_d.ap().rearrange("p (i c) -> p i c", c=C)      # [128, N1, C]
    xh_v = xh8_d.ap().rearrange("p (g n) -> p g n", n=N)      # [128, G, N]
    w_v = w8_d.ap().rearrange("p (t g o) -> p t g o", t=3, o=C)
    y_v = y_d.ap().rearrange("p (g n) -> p g n", n=N)

    MUL = mybir.AluOpType.mult
    ADD = mybir.AluOpType.add
    MIN = mybir.AluOpType.min
    Exp = mybir.ActivationFunctionType.Exp
    Sqrt = mybir.ActivationFunctionType.Sqrt
    Rsqrt = mybir.ActivationFunctionType.Rsqrt
    Copy = mybir.ActivationFunctionType.Copy

    with tile.TileContext(nc) as tc:
        with (
            tc.tile_pool(name="consts", bufs=1) as cpool,
            tc.tile_pool(name="weights", bufs=1) as wpool,
            tc.tile_pool(name="xbig", bufs=1) as xpool,
            tc.tile_pool(name="inter8", bufs=1) as ipool,
            tc.tile_pool(name="norm", bufs=1) as npool,
            tc.tile_pool(name="smx", bufs=2) as spool,
            tc.tile_pool(name="small", bufs=2) as qpool,
            tc.tile_pool(name="outs", bufs=2) as opool,
            tc.tile_pool(name="ps", bufs=1, space="PSUM") as ps,
        ):
            # one PSUM tile per bank; phases reuse banks via slices with
            # natural data-dependency slack.
            bk = [ps.tile([128, 512], FP32, tag=f"bk{i}", name=f"bk{i}")
                  for i in range(8)]

            # ---- scratch + ACT table preloads (run during DMA wait) ----
            scr = cpool.tile([1, 4], FP32, tag="scr")
            warm8 = None
            with tc.high_priority():
                if F_WARM:
                    warm8 = cpool.tile([128, 2, 512], F8, tag="warm8")
                    nc.vector.memset(warm8[:], 0.0)
                nc.gpsimd.memset(scr[:], 1.0)
                if F_RSQ:
                    _act_raw(nc, scr[:, 1:2], scr[:, 0:1], Rsqrt)
                else:
                    nc.scalar.activation(scr[:, 1:2], scr[:, 0:1], Sqrt)
                _act_raw(nc, scr[:, 2:3], scr[:, 0:1], Exp)

            # ---- input DMAs: xt8 first (Gx critical path) --------------
            xt8 = xpool.tile([128, N1, C], F8, tag="xt8")
            for s in range(8):
                nc.sync.dma_start(xt8[:, 4 * s:4 * s + 4, :],
                                  xt_v[:, 4 * s:4 * s + 4, :])
            w8 = wpool.tile([128, 3, G, C], F8, tag="w8")
            nc.sync.dma_start(w8[:], w_v)
            wq8, wk8, wv8 = w8[:, 0], w8[:, 1], w8[:, 2]
            gamma_col = cpool.tile([128, 1], FP32, tag="gamma_col")
            nc.sync.dma_start(gamma_col[:], gcol_d.ap())
            ones_bf = cpool.tile([128, 1], BF16, tag="ones_bf")
            nc.sync.dma_start(ones_bf[:], obf_d.ap())
            ones_row = cpool.tile([1, 128], BF16, tag="ones_row")
            nc.sync.dma_start(ones_row[:], orow_d.ap())
            ones8 = cpool.tile([128, 1], F8, tag="ones8")
            nc.sync.dma_start(ones8[:], o8_d.ap())
            xh8 = xpool.tile([128, G, N], F8, tag="xh8")
            nc.sync.dma_start(xh8[:, 0:2, :], xh_v[:, 0:2, :])
            nc.sync.dma_start(xh8[:, 2:4, :], xh_v[:, 2:4, :])

            # ---- PE warm-up during DMA wait (bank 7, garbage output) ---
            if F_WARM:
                for w in range(8):
                    nc.tensor.matmul(bk[7][:], warm8[:, :, 0:128],
                                     warm8[:], start=True, stop=True,
                                     perf_mode=DR)

            # ---- Gx = X X^T (fp8 DR, PSUM-accumulated) banks 0-3 -------
            gx8 = ipool.tile([128, G, C], F8, tag="gx8")
            for i in range(NP):
                for cg in range(G):
                    nc.tensor.matmul(bk[cg][:],
                                     xt8[:, 2 * i:2 * i + 2,
                                         cg * 128:(cg + 1) * 128],
                                     xt8[:, 2 * i:2 * i + 2, :],
                                     start=(i == 0), stop=(i == NP - 1),
                                     perf_mode=DR)
            for cg in range(G):
                if cg % 2:
                    nc.vector.tensor_scalar(gx8[:, cg, :], bk[cg][:],
                                            1.0 / 64.0, None, op0=MUL)
                else:
                    nc.scalar.activation(gx8[:, cg, :], bk[cg][:],
                                         Copy, scale=1.0 / 64.0)

            # ---- M3q = S Wq^T (banks 4-7), M3k = S Wk^T (banks 0-3) ----
            m3q8 = ipool.tile([128, G, C], F8, tag="m3q8")
            m3k8 = None
            if not F_STT:
                m3k8 = ipool.tile([128, G, C], F8, tag="m3k8")
            # interleave M3q pair / q-cast / M3k pair per cg so each cast
            # fires right after its producer in the PE stream
            for cg in range(G):
                for p in range(2):
                    nc.tensor.matmul(bk[4 + cg][:],
                                     gx8[:, 2 * p:2 * p + 2,
                                         cg * 128:(cg + 1) * 128],
                                     wq8[:, 2 * p:2 * p + 2, :],
                                     start=(p == 0), stop=(p == 1),
                                     perf_mode=DR)
                if cg % 2:
                    nc.vector.tensor_scalar(m3q8[:, cg, :], bk[4 + cg][:],
                                            0.5, None, op0=MUL)
                else:
                    nc.scalar.activation(m3q8[:, cg, :], bk[4 + cg][:],
                                         Copy, scale=0.5)
                for p in range(2):
                    nc.tensor.matmul(bk[cg][:],
                                     gx8[:, 2 * p:2 * p + 2,
                                         cg * 128:(cg + 1) * 128],
                                     wk8[:, 2 * p:2 * p + 2, :],
                                     start=(p == 0), stop=(p == 1),
                                     perf_mode=DR)

            # ---- norms: tq/tk = W8 o M3 (2|Q|^2 / 2|K|^2 colsums) ------
            # tk fused from k_ps PSUM via scalar_tensor_tensor (no m3k8
            # fp8 cast at all); tq g<2 from q_ps PSUM, g>=2 from m3q8.
            tq = npool.tile([128, G, C], BF16, tag="tq")
            tk = npool.tile([128, G, C], BF16, tag="tk")
            for g in range(G):
                if F_STT and g < 2:
                    nc.vector.scalar_tensor_tensor(
                        tq[:, g, :], bk[4 + g][:], 0.5, wq8[:, g, :],
                        op0=MUL, op1=MUL)
                elif F_GPS_TT and g == 3:
                    nc.gpsimd.tensor_tensor(tq[:, g, :], wq8[:, g, :],
                                            m3q8[:, g, :], op=MUL)
                else:
                    nc.vector.tensor_tensor(tq[:, g, :], wq8[:, g, :],
                                            m3q8[:, g, :], op=MUL)
            for g in range(G):
                if F_STT:
                    nc.vector.scalar_tensor_tensor(
                        tk[:, g, :], bk[g][:], 0.5, wk8[:, g, :],
                        op0=MUL, op1=MUL)
                else:
                    nc.vector.tensor_scalar(m3k8[:, g, :], bk[g][:],
                                            0.5, None, op0=MUL)
                    nc.vector.tensor_tensor(tk[:, g, :], wk8[:, g, :],
                                            m3k8[:, g, :], op=MUL)

            # ---- G^T per d-group (banks 4-7, alive through softmax) ----
            # dg 2,3 first: their banks have no pending tq readers.
            for dg in (2, 3, 0, 1):
                for p in range(2):
                    nc.tensor.matmul(
                        bk[4 + dg][:],
                        wk8[:, 2 * p:2 * p + 2, dg * 128:(dg + 1) * 128],
                        m3q8[:, 2 * p:2 * p + 2, :],
                        start=(p == 0), stop=(p == 1), perf_mode=DR)

            # sqq = colsum(tq) -> bank 0 row 0 (free axis = q-channel)
            for g in range(G):
                nc.tensor.matmul(bk[0][0:1, :], ones_bf[:], tq[:, g, :],
                                 start=(g == 0), stop=(g == G - 1))
            # sqk[dg] -> banks 1,2,3 col 0 and bank 0 col 4
            sqk_bank = [1, 2, 3, 0]
            sqk_col = [0, 0, 0, 4]
            for dg in range(G):
                b, cl = sqk_bank[dg], sqk_col[dg]
                for g in range(G):
                    nc.tensor.matmul(bk[b][:, cl:cl + 1],
                                     tk[:, g, dg * 128:(dg + 1) * 128],
                                     ones_bf[:],
                                     start=(g == 0), stop=(g == G - 1))

            # rq = 0.5/|Q| = Rsqrt(2*sqq); rk = 1/|K| = Rsqrt(0.5*sqk)
            rq_bf = npool.tile([1, C], BF16, tag="rq_bf")
            rk4 = npool.tile([128, 4], FP32, tag="rk4")
            if F_RSQ:
                _act_raw(nc, rq_bf[:], bk[0][0:1, :], Rsqrt, scale=2.0)
                for dg in range(G):
                    b, cl = sqk_bank[dg], sqk_col[dg]
                    _act_raw(nc, rk4[:, dg:dg + 1], bk[b][:, cl:cl + 1],
                             Rsqrt, scale=0.5)
            else:
                invq = npool.tile([1, C], FP32, tag="invq")
                nc.vector.reciprocal(invq[:], bk[0][0:1, :])
                nc.scalar.activation(rq_bf[:], invq[:], Sqrt, scale=0.5)
                invk4 = npool.tile([128, 4], FP32, tag="invk4")
                for dg in range(G):
                    b, cl = sqk_bank[dg], sqk_col[dg]
                    nc.vector.reciprocal(invk4[:, dg:dg + 1],
                                         bk[b][:, cl:cl + 1])
                nc.scalar.activation(rk4[:], invk4[:], Sqrt, scale=2.0)
            nrk4 = npool.tile([128, 4], FP32, tag="nrk4")
            nc.vector.tensor_scalar(nrk4[:], rk4[:], -1.0, None, op0=MUL)
            rk4h = npool.tile([128, 4], FP32, tag="rk4h")
            nc.vector.tensor_scalar(rk4h[:], rk4[:], INV_H, None, op0=MUL)

            # pinned dummy exp: forces the Exp table load into the ACT
            # idle window before the real exps need it
            with tc.tile_wait_until(0.024):
                _act_raw(nc, scr[:, 3:4], scr[:, 0:1], Exp)

            # bq = broadcast(rq) via ones-column matmul -> bank 1
            nc.tensor.matmul(bk[1][:], ones_row[:], rq_bf[:],
                             start=True, stop=True)
            bq = npool.tile([128, C], BF16, tag="bq")
            nc.scalar.copy(bq[:], bk[1][:])

            # filler matmul: keep HAM warm between bq and A^T (bank 2)
            if F_WARM:
                nc.tensor.matmul(bk[2][:], warm8[:, :, 0:128], warm8[:],
                                 start=True, stop=True, perf_mode=DR)

            # ---- softmax chains on G^T[d,c] + A^T accumulation ---------
            msm8 = ipool.tile([128, G, C], F8, tag="msm8")
            at8 = ipool.tile([128, G, C], F8, tag="at8")
            at_banks = [2, 3, 0, 1]
            for dg in range(G):
                t1 = spool.tile([128, C], BF16, tag="t1")
                mn = qpool.tile([128, 1], FP32, tag="mn")
                if F_TTR:
                    nc.vector.tensor_tensor_reduce(
                        t1[:], bq[:], bk[4 + dg][:], 1.0, 0.0,
                        op0=MUL, op1=MIN, accum_out=mn[:])
                else:
                    nc.vector.tensor_tensor(t1[:], bk[4 + dg][:], bq[:],
                                            op=MUL)
                    nc.vector.tensor_reduce(mn[:], t1[:],
                                            axis=mybir.AxisListType.X, op=MIN)
                den = qpool.tile([128, 1], FP32, tag="den")
                eng = nc.gpsimd if F_GPS_APS else nc.vector
                eng.tensor_scalar(den[:], mn[:], nrk4[:, dg:dg + 1],
                                  1.0 + EPS, op0=MUL, op1=ADD)
                r = qpool.tile([128, 1], FP32, tag="r")
                nc.vector.reciprocal(r[:], den[:])
                sv = qpool.tile([128, 1], FP32, tag="sv")
                eng.tensor_scalar(sv[:], r[:], rk4h[:, dg:dg + 1],
                                  None, op0=MUL)
                bv = qpool.tile([128, 1], FP32, tag="bv")
                nc.gpsimd.tensor_scalar(bv[:], r[:], -INV_H, 1.0,
                                        op0=MUL, op1=ADD)
                e = spool.tile([128, C], BF16, tag="e")
                se = qpool.tile([128, 1], FP32, tag="se")
                nc.scalar.activation(e[:], t1[:], Exp,
                                     bias=bv[:], scale=sv[:],
                                     accum_out=se[:])
                rd = qpool.tile([128, 1], FP32, tag="rd")
                nc.vector.reciprocal(rd[:], se[:])
                if dg % 2:
                    # ACT path: msm = Copy(e * rd64), rd64 on gpsimd
                    rd64 = qpool.tile([128, 1], FP32, tag="rd64")
                    nc.gpsimd.tensor_scalar(rd64[:], rd[:], 64.0, None,
                                            op0=MUL)
                    nc.scalar.activation(msm8[:, dg, :], e[:], Copy,
                                         scale=rd64[:])
                else:
                    nc.vector.tensor_scalar(msm8[:, dg, :], e[:], rd[:],
                                            64.0, op0=MUL, op1=MUL)
                if F_WARM and dg == 0:
                    nc.tensor.matmul(bk[3][:], wv8[:, 0, 0:128],
                                     msm8[:, 0, :], start=True, stop=True)
                # A^T accumulation over dg pairs into banks 2,3,0,1
                if dg % 2:
                    p = dg // 2
                    for eg in range(G):
                        nc.tensor.matmul(
                            bk[at_banks[eg]][:],
                            wv8[:, dg - 1:dg + 1,
                                eg * 128:(eg + 1) * 128],
                            msm8[:, dg - 1:dg + 1, :],
                            start=(p == 0), stop=(p == 1), perf_mode=DR)

            # ---- row-L1 norm -> per-channel scale fcol (banks 4-7) -----
            # rsum[c] = 64 * sum_d Msm[c,d]; fcol = gamma / (32*(rsum+64eps))
            for cg in range(G):
                for g in range(G):
                    nc.tensor.matmul(bk[4 + cg][:, 0:1],
                                     msm8[:, g, cg * 128:(cg + 1) * 128],
                                     ones8[:],
                                     start=(g == 0), stop=(g == G - 1))
            eps4 = npool.tile([128, 4], FP32, tag="eps4")
            for cg in range(G):
                nc.vector.tensor_scalar(eps4[:, cg:cg + 1],
                                        bk[4 + cg][:, 0:1],
                                        64.0 * EPS, None, op0=ADD)
            invr4 = npool.tile([128, 4], FP32, tag="invr4")
            nc.vector.reciprocal(invr4[:], eps4[:])
            fcol4 = npool.tile([128, 4], FP32, tag="fcol4")
            eng = nc.gpsimd if F_GPS_APS else nc.vector
            eng.tensor_scalar(fcol4[:], invr4[:], gamma_col[:],
                              None, op0=MUL)

            # at8 = 2 * at_ps (fp8 cast; scale keeps absmax ~60)
            for eg in range(G):
                if eg % 2:
                    nc.vector.tensor_scalar(at8[:, eg, :],
                                            bk[at_banks[eg]][:],
                                            2.0, None, op0=MUL)
                else:
                    nc.scalar.activation(at8[:, eg, :], bk[at_banks[eg]][:],
                                         Copy, scale=2.0)

            # ---- phase 2: r = (A X) * fcol, bf16 store -----------------
            for cg in range(G):
                ofin = opool.tile([128, N], F8, tag="ofin",
                                  name=f"ofin{cg}")
                for p in range(2):
                    lhs = at8[:, 2 * p:2 * p + 2,
                              cg * 128:(cg + 1) * 128]
                    for j in range(NJ):
                        nc.tensor.matmul(bk[j][:], lhs,
                                         xh8[:, 2 * p:2 * p + 2,
                                             j * 512:(j + 1) * 512],
                                         start=(p == 0), stop=(p == 1),
                                         perf_mode=DR)
                for j in range(NJ):
                    jsl = slice(j * 512, (j + 1) * 512)
                    if j % 2:
                        nc.vector.tensor_scalar(ofin[:, jsl], bk[j][:],
                                                fcol4[:, cg:cg + 1],
                                                None, op0=MUL)
                    else:
                        nc.scalar.activation(ofin[:, jsl], bk[j][:],
                                             Copy, scale=fcol4[:, cg:cg + 1])
                    if j == 3:
                        nc.sync.dma_start(y_v[:, cg, 0:2048],
                                          ofin[:, 0:2048])
                    if cg == G - 1 and j == 5:
                        nc.sync.dma_start(y_v[:, cg, 2048:3072],
                                          ofin[:, 2048:3072])
                if cg == G - 1:
                    nc.sync.dma_start(y_v[:, cg, 3072:4096],
                                      ofin[:, 3072:4096])
                else:
                    nc.sync.dma_start(y_v[:, cg, 2048:4096],
                                      ofin[:, 2048:4096])

    nc.compile()
    return nc


def _get_nc():
    if "nc" not in _CACHE:
        _CACHE["nc"] = _build_nc()
    return _CACHE["nc"]


def _pack_rows(a):
    """[C, M] -> [128, G*M] with row r = g*128+p at [p, g*M:(g+1)*M]."""
    Cr, M = a.shape
    return np.ascontiguousarray(
        a.reshape(Cr // 128, 128, M).transpose(1, 0, 2).reshape(128, -1))


def _make_in_maps(x, Wq, Wk, Wv, gamma):
    F8NP = ml_dtypes.float8_e4m3
    xb = np.ascontiguousarray(x.reshape(B, C, N).astype(np.float32))
    xh8 = xb.astype(F8NP)
    xt8 = np.ascontiguousarray(xb.transpose(0, 2, 1)).astype(F8NP)
    # pack to [128, *] SBUF layouts (one contiguous line per partition)
    xt8p = np.ascontiguousarray(
        xt8.reshape(B, N1, 128, C).transpose(0, 2, 1, 3).reshape(B, 128, -1))
    xh8p = np.ascontiguousarray(
        xh8.reshape(B, G, 128, N).transpose(0, 2, 1, 3).reshape(B, 128, -1))
    wq8 = _pack_rows((np.ascontiguousarray(Wq.T) * 16.0).astype(F8NP))
    wk8 = _pack_rows((np.ascontiguousarray(Wk.T) * 16.0).astype(F8NP))
    wv8 = _pack_rows((np.asarray(Wv) * 16.0).astype(F8NP))
    w8 = np.ascontiguousarray(np.concatenate([wq8, wk8, wv8], axis=1))
    gval = float(np.asarray(gamma).reshape(-1)[0])
    gcol = np.full((128, 1), gval / 2.0, np.float32)  # gamma/32 * 16 (fp8 out scale)
    obf = np.ones((128, 1), ml_dtypes.bfloat16)
    orow = np.ones((1, 128), ml_dtypes.bfloat16)
    o8 = np.ones((128, 1), F8NP)
    maps = []
    for i in range(B):
        maps.append({
            "xt8": xt8p[i], "xh8": xh8p[i], "w8": w8,
            "gamma_col": gcol, "ones_bf": obf, "ones_row": orow,
            "ones_f8": o8,
        })
    return maps


def kernel(x, Wq, Wk, Wv, gamma, _trace=False, _trace_kwargs=None):
    nc = _get_nc()
    xnp = np.asarray(x)
    in_maps = _make_in_maps(xnp, np.asarray(Wq), np.asarray(Wk),
                            np.asarray(Wv), np.asarray(gamma))
    kwargs = {}
    if _trace:
        kwargs = dict(trace=True, **(_trace_kwargs or {}))
    res = bass_utils.run_bass_kernel_spmd(nc, in_maps,
                                          core_ids=list(range(B)), **kwargs)
    # y [128, G*N] -> [C, N]
    r = np.stack([
        res.results[i]["y"].reshape(128, G, N).transpose(1, 0, 2)
        .reshape(C, N).astype(np.float32) for i in range(B)]) * (1.0 / 16.0)
    y = xnp.reshape(B, C, N).astype(np.float32) + r
    if _trace:
        kernel._last_result = res
    return y.reshape(B, C, HH, WW).astype(np.float32)
